# revision 1
# baseline (speedup 1.0000x reference)
"""Trainium2 Bass kernel for nn_Deep_Pron (sparse_attention).

Pipeline per core (N-sharded: 4 speakers/core):
  Phase 1: stream X1,X2; per-channel sum/sumsq (BN2d stats) -> AllReduce.
  Phase 1.5: BN2d affine coefs s,t per channel.
  Phase 2: re-stream X + masks; BN-apply (ACT); quadform S via PE
    transpose chunks + blockdiag eigen-matmul + square + blockdiag +/-
    reduce (S lands pair-major [P,100]); softmax; attention output h via
    Pool broadcast-mul + DVE segmented reduce; feats = log||h1-h2||^2.
  Phase 2.5: BN1d stats AllReduce; BN1d apply.
  Phase 3: 7-layer MLP on PE; output y[4] per core.
"""

import numpy as np

N, D, V, NF = 32, 1128, 100, 13
H = 1000
EPS = 1e-5
NCORES = 8
NSPK = N // NCORES  # 4
CHS = [128] * 8 + [104]  # d-chunks per speaker
NCH = len(CHS)
# transpose sub-chunks over the (v,f)=1300 free dim: 11x(9v=117 cols) + 1x(1v=13)
TCH = [(cc * 117, 117, 9) for cc in range(11)] + [(1287, 13, 1)]
CNT2D = float(N * V * NF)  # BN2d count
HP = 1024  # padded H
DP = 1152  # padded D


def _host_prep(attn_w, bn2d_gamma, bn2d_beta, bn1_gamma, bn1_beta, fcs):
    """Build all constant tensors (numpy, f32)."""
    Asym = ((attn_w.T + attn_w) / 2.0).astype(np.float64)
    lam, Q = np.linalg.eigh(Asym)
    B = (Q * np.sqrt(np.abs(lam))[None, :]).astype(np.float32)  # [13,13]
    sign = np.where(lam >= 0, 1.0, -1.0).astype(np.float32)

    bdz = np.zeros((117, 117), np.float32)
    bds = np.zeros((117, 9), np.float32)
    for vp in range(9):
        bdz[13 * vp:13 * vp + 13, 13 * vp:13 * vp + 13] = B
        bds[13 * vp:13 * vp + 13, vp] = sign

    ident = np.eye(128, dtype=np.float32)

    def chunkmajor(vec, pad_val):
        out = np.full((128, NCH), pad_val, np.float32)
        for c, P in enumerate(CHS):
            out[:P, c] = vec[128 * c:128 * c + P]
        return out

    bn2g = chunkmajor(bn2d_gamma, 1.0)
    bn2b = chunkmajor(bn2d_beta, 0.0)
    bn1g = chunkmajor(bn1_gamma, 1.0)
    bn1b = chunkmajor(bn1_beta, 0.0)

    (f1w, f1b, f2w, f2b, f3w, f3b, f4w, f4b, f5w, f5b, f6w, f6b, f7w, f7b) = fcs
    w1t = np.zeros((DP, HP), np.float32)
    w1t[:D, :H] = f1w.T  # [D,H]
    wts = [w1t]
    for w in (f2w, f3w, f4w, f5w, f6w):
        wt = np.zeros((HP, HP), np.float32)
        wt[:H, :H] = w.T
        wts.append(wt)
    w7t = np.zeros((HP, 1), np.float32)
    w7t[:H, 0] = f7w[0]
    biases = []
    for b in (f1b, f2b, f3b, f4b, f5b, f6b):
        bb = np.zeros((128, 8), np.float32)
        for j in range(8):
            seg = b[128 * j:128 * j + 128]
            bb[:len(seg), j] = seg
        biases.append(bb)
    return bdz, bds, ident, bn2g, bn2b, bn1g, bn1b, wts, w7t, biases, float(f7b[0])


def _build_nc(b7_val, level=99):
    import concourse.bass as bass
    import concourse.bacc as bacc
    import concourse.mybir as mybir
    import concourse.tile as tile

    dt = mybir.dt.float32
    Alu = mybir.AluOpType
    Act = mybir.ActivationFunctionType
    Ax = mybir.AxisListType

    nc = bacc.Bacc("TRN2", target_bir_lowering=False, debug=True)

    def din(name, shape):
        return nc.declare_dram_parameter(name, list(shape), dt, isOutput=False)

    x1 = din("x1", (NSPK, D, V * NF))
    x2 = din("x2", (NSPK, D, V * NF))
    m1 = din("m1", (NSPK, D, V * NF))
    m2 = din("m2", (NSPK, D, V * NF))
    bdz_d = din("bdz", (117, 117))
    bds_d = din("bds", (117, 9))
    id_d = din("ident", (128, 128))
    bn2g_d = din("bn2g", (128, NCH))
    bn2b_d = din("bn2b", (128, NCH))
    bn1g_d = din("bn1g", (128, NCH))
    bn1b_d = din("bn1b", (128, NCH))
    w_d = [din(f"w{l}t", (DP if l == 1 else HP, HP)) for l in range(1, 7)]
    w7_d = din("w7t", (HP, 1))
    b_d = [din(f"b{l}", (128, 8)) for l in range(1, 7)]
    y_out = nc.declare_dram_parameter("y", [1, NSPK], dt, isOutput=True)

    xs = (x1, x2)
    ms = (m1, m2)

    with tile.TileContext(nc) as tc:
        with (
            tc.tile_pool(name="singles", bufs=1) as singles,
            tc.tile_pool(name="xin", bufs=3) as xin_pool,
            tc.tile_pool(name="min", bufs=3) as min_pool,
            tc.tile_pool(name="xhat", bufs=2) as xhat_pool,
            tc.tile_pool(name="xt", bufs=4) as xt_pool,
            tc.tile_pool(name="zsq", bufs=4) as zsq_pool,
            tc.tile_pool(name="sm", bufs=4) as sm_pool,
            tc.tile_pool(name="tiny", bufs=8) as tiny_pool,
            tc.tile_pool(name="scratch", bufs=2) as scr_pool,
            tc.tile_pool(name="wpool", bufs=10) as w_pool,
            tc.tile_pool(name="tp_ps", bufs=2, space="PSUM") as tp_ps,
            tc.tile_pool(name="z_ps", bufs=2, space="PSUM") as z_ps,
            tc.tile_pool(name="s_ps", bufs=2, space="PSUM") as s_ps,
            tc.tile_pool(name="mlp_ps", bufs=1, space="PSUM") as mlp_ps,
            tc.tile_pool(name="dram", bufs=1, space="DRAM") as dram,
        ):
            # --- resident constants ---
            ident = singles.tile([128, 128], dt)
            nc.sync.dma_start(ident[:], id_d[:])
            bdz = singles.tile([128, 117], dt)
            nc.sync.dma_start(bdz[:117, :], bdz_d[:])
            bds = singles.tile([128, 9], dt)
            nc.sync.dma_start(bds[:117, :], bds_d[:])
            bn2g = singles.tile([128, NCH], dt)
            nc.sync.dma_start(bn2g[:], bn2g_d[:])
            bn2b = singles.tile([128, NCH], dt)
            nc.sync.dma_start(bn2b[:], bn2b_d[:])
            bn1g = singles.tile([128, NCH], dt)
            nc.sync.dma_start(bn1g[:], bn1g_d[:])
            bn1b = singles.tile([128, NCH], dt)
            nc.sync.dma_start(bn1b[:], bn1b_d[:])

            # --- phase 1: BN2d stats ---
            # acc[xsel]: sum, sumsq tiles [128, NCH]
            acc_sum = [singles.tile([128, NCH], dt, tag=f"acs{i}", name=f"acs{i}") for i in range(2)]
            acc_sq = [singles.tile([128, NCH], dt, tag=f"acq{i}", name=f"acq{i}") for i in range(2)]
            for t in (*acc_sum, *acc_sq):
                nc.vector.memset(t[:], 0.0)

            for n in range(NSPK):
                for c, P in enumerate(CHS):
                    for xi in range(2):
                        xt_ = xin_pool.tile([128, V * NF], dt, tag="p1x", name="p1x")
                        nc.sync.dma_start(
                            xt_[:P, :], xs[xi][n, 128 * c:128 * c + P, :])
                        part = tiny_pool.tile([128, 1], dt, tag="p1part", name="p1part")
                        nc.vector.tensor_reduce(
                            part[:P, :], xt_[:P, :], axis=Ax.X, op=Alu.add)
                        nc.vector.tensor_tensor(
                            acc_sum[xi][:P, c:c + 1], acc_sum[xi][:P, c:c + 1],
                            part[:P, :], op=Alu.add)
                        sq = scr_pool.tile([128, V * NF], dt, tag="p1sq", name="p1sq")
                        sqp = tiny_pool.tile([128, 1], dt, tag="p1sqp", name="p1sqp")
                        nc.scalar.activation(
                            sq[:P, :], xt_[:P, :], Act.Square,
                            accum_out=sqp[:P, :])
                        nc.vector.tensor_tensor(
                            acc_sq[xi][:P, c:c + 1], acc_sq[xi][:P, c:c + 1],
                            sqp[:P, :], op=Alu.add)

            # all-reduce the 4 stat tiles
            st_in = dram.tile([128, 4 * NCH], dt, tag="st_in", name="st_in")
            st_out = dram.tile([128, 4 * NCH], dt, tag="st_out", name="st_out")
            for i in range(2):
                nc.sync.dma_start(st_in[:, NCH * i:NCH * (i + 1)], acc_sum[i][:])
                nc.sync.dma_start(
                    st_in[:, NCH * (2 + i):NCH * (3 + i)], acc_sq[i][:])
            nc.gpsimd.collective_compute(
                "AllReduce", mybir.AluOpType.add,
                replica_groups=[list(range(NCORES))],
                ins=[st_in[:].opt()], outs=[st_out[:].opt()])
            stats = singles.tile([128, 4 * NCH], dt)
            nc.sync.dma_start(stats[:], st_out[:])

            # --- phase 1.5: per-channel affine coefs  s=g*rsqrt(var+eps), t=b-mean*s
            s_co = [singles.tile([128, NCH], dt, tag=f"sco{i}", name=f"sco{i}") for i in range(2)]
            t_co = [singles.tile([128, NCH], dt, tag=f"tco{i}", name=f"tco{i}") for i in range(2)]
            for i in range(2):
                mean = tiny_pool.tile([128, NCH], dt, tag="mean", name="mean")
                nc.vector.tensor_scalar_mul(
                    mean[:], stats[:, NCH * i:NCH * (i + 1)], 1.0 / CNT2D)
                msq = tiny_pool.tile([128, NCH], dt, tag="msq", name="msq")
                nc.scalar.activation(msq[:], mean[:], Act.Square)
                var = tiny_pool.tile([128, NCH], dt, tag="var", name="var")
                nc.vector.tensor_scalar_mul(
                    var[:], stats[:, NCH * (2 + i):NCH * (3 + i)], 1.0 / CNT2D)
                nc.vector.tensor_tensor(var[:], var[:], msq[:], op=Alu.subtract)
                nc.vector.tensor_scalar_add(var[:], var[:], EPS)
                sd = tiny_pool.tile([128, NCH], dt, tag="sd", name="sd")
                nc.scalar.activation(sd[:], var[:], Act.Sqrt)
                rs = tiny_pool.tile([128, NCH], dt, tag="rs", name="rs")
                nc.vector.reciprocal(rs[:], sd[:])
                nc.vector.tensor_tensor(s_co[i][:], rs[:], bn2g[:], op=Alu.mult)
                tm = tiny_pool.tile([128, NCH], dt, tag="tm", name="tm")
                nc.vector.tensor_tensor(tm[:], mean[:], s_co[i][:], op=Alu.mult)
                nc.vector.tensor_tensor(t_co[i][:], bn2b[:], tm[:], op=Alu.subtract)

            # --- phase 2: attention + feats ---
            featsT = singles.tile([128, NCH * NSPK], dt)  # col = c*NSPK+n
            nc.vector.memset(featsT[:], 0.0)

            for n in range(NSPK):
                for c, P in enumerate(CHS):
                    hraw = [None, None]
                    m00 = [None, None]
                    for xi in range(2):
                        xnat = xin_pool.tile([128, V * NF], dt, tag="p2x", name="p2x")
                        nc.sync.dma_start(
                            xnat[:P, :], xs[xi][n, 128 * c:128 * c + P, :])
                        mnat = min_pool.tile([128, V * NF], dt, tag="p2m", name="p2m")
                        nc.sync.dma_start(
                            mnat[:P, :], ms[xi][n, 128 * c:128 * c + P, :])
                        # BN apply
                        xh = xhat_pool.tile([128, V * NF], dt, tag="xh", name="xh")
                        nc.scalar.activation(
                            xh[:P, :], xnat[:P, :], Act.Identity,
                            bias=t_co[xi][:P, c:c + 1], scale=s_co[xi][:P, c:c + 1])
                        # quadform: S pair-major [P, 100]
                        s_psum = s_ps.tile([128, V], dt, tag="spsum", name="spsum")
                        for (off, W, Vc) in TCH:
                            tp = tp_ps.tile([128, 128], dt, tag="tp", name="tp")
                            nc.tensor.transpose(
                                tp[:W, :P], xh[:P, off:off + W], ident[:P, :P])
                            xts = xt_pool.tile([128, 128], dt, tag="xts", name="xts")
                            nc.vector.tensor_copy(xts[:W, :P], tp[:W, :P])
                            zp = z_ps.tile([128, 128], dt, tag="zp", name="zp")
                            nc.tensor.matmul(
                                zp[:W, :P], bdz[:W, :W], xts[:W, :P],
                                start=True, stop=True)
                            zq = zsq_pool.tile([128, 128], dt, tag="zq", name="zq")
                            nc.scalar.activation(zq[:W, :P], zp[:W, :P], Act.Square)
                            vo = off // 13 // 9 * 9
                            nc.tensor.matmul(
                                s_psum[:P, vo:vo + Vc], zq[:W, :P], bds[:W, :Vc],
                                start=True, stop=True)
                        # logits = tanh(S) + 1e5*m0 - 1e5
                        tanh_s = sm_pool.tile([128, V], dt, tag="tanhs", name="tanhs")
                        nc.scalar.activation(
                            tanh_s[:P, :], s_psum[:P, :V], Act.Tanh)
                        mterm = sm_pool.tile([128, V], dt, tag="mterm", name="mterm")
                        m0view = mnat[:P].rearrange("p (v f) -> p v f", f=NF)
                        nc.scalar.activation(
                            mterm[:P, :], m0view[:, :, 0], Act.Copy,
                            scale=1.0e5, bias=-1.0e5)
                        logits = sm_pool.tile([128, V], dt, tag="logits", name="logits")
                        nc.vector.tensor_tensor(
                            logits[:P, :], tanh_s[:P, :], mterm[:P, :], op=Alu.add)
                        # softmax
                        mx = tiny_pool.tile([128, 1], dt, tag="mx", name="mx")
                        nc.vector.tensor_reduce(
                            mx[:P, :], logits[:P, :], axis=Ax.X, op=Alu.max)
                        nmx = tiny_pool.tile([128, 1], dt, tag="nmx", name="nmx")
                        nc.vector.tensor_scalar_mul(nmx[:P, :], mx[:P, :], -1.0)
                        esum = tiny_pool.tile([128, 1], dt, tag="esum", name="esum")
                        ew = sm_pool.tile([128, V], dt, tag="ew", name="ew")
                        nc.scalar.activation(
                            ew[:P, :], logits[:P, :], Act.Exp,
                            bias=nmx[:P, :], accum_out=esum[:P, :])
                        winv = tiny_pool.tile([128, 1], dt, tag="winv", name="winv")
                        nc.vector.reciprocal(winv[:P, :], esum[:P, :])
                        wl3 = sm_pool.tile([128, V], dt, tag="wl3", name="wl3")
                        nc.vector.tensor_scalar_mul(wl3[:P, :], ew[:P, :], winv[:P, :])
                        # h_raw[i] = sum_v W[v] * x[v,i]  (raw x)
                        pall = scr_pool.tile([128, V * NF], dt, tag="pall", name="pall")
                        wb = (wl3[:P, :].rearrange("p (v o) -> p v o", o=1)
                              .broadcast_to((P, V, NF)))
                        xv = xnat[:P].rearrange("p (v f) -> p v f", f=NF)
                        pv = pall[:P].rearrange("p (v f) -> p v f", f=NF)
                        nc.gpsimd.tensor_tensor(pv, xv, wb, op=Alu.mult)
                        hr = tiny_pool.tile([128, NF], dt, tag=f"hr{xi}", name=f"hr{xi}")
                        nc.vector.tensor_reduce(
                            hr[:P, :], pall[:P].rearrange("p (v f) -> p f v", f=NF),
                            axis=Ax.X, op=Alu.add)
                        hraw[xi] = hr
                        mm = tiny_pool.tile([128, 1], dt, tag=f"m00{xi}", name=f"m00{xi}")
                        nc.vector.tensor_copy(mm[:P, :], mnat[:P, 0:1])
                        m00[xi] = mm
                    # feats: g_i = s1*h1_i - s2*h2_i + (t1-t2);  dd = sum g^2
                    g1 = tiny_pool.tile([128, NF], dt, tag="g1", name="g1")
                    nc.vector.tensor_scalar(
                        g1[:P, :], hraw[0][:P, :], s_co[0][:P, c:c + 1],
                        t_co[0][:P, c:c + 1], op0=Alu.mult, op1=Alu.add)
                    g2 = tiny_pool.tile([128, NF], dt, tag="g2", name="g2")
                    nc.vector.tensor_scalar(
                        g2[:P, :], hraw[1][:P, :], s_co[1][:P, c:c + 1],
                        t_co[1][:P, c:c + 1], op0=Alu.mult, op1=Alu.add)
                    gd = tiny_pool.tile([128, NF], dt, tag="gd", name="gd")
                    nc.vector.tensor_tensor(
                        gd[:P, :], g1[:P, :], g2[:P, :], op=Alu.subtract)
                    gsq = tiny_pool.tile([128, NF], dt, tag="gsq", name="gsq")
                    dd = tiny_pool.tile([128, 1], dt, tag="dd", name="dd")
                    nc.scalar.activation(
                        gsq[:P, :], gd[:P, :], Act.Square, accum_out=dd[:P, :])
                    nc.vector.tensor_scalar_add(dd[:P, :], dd[:P, :], EPS)
                    lg = tiny_pool.tile([128, 1], dt, tag="lg", name="lg")
                    nc.scalar.activation(lg[:P, :], dd[:P, :], Act.Ln)
                    pm = tiny_pool.tile([128, 1], dt, tag="pm", name="pm")
                    nc.vector.tensor_tensor(
                        pm[:P, :], m00[0][:P, :], m00[1][:P, :], op=Alu.mult)
                    # feats = (lg+1)*pm - 1
                    lp1 = tiny_pool.tile([128, 1], dt, tag="lp1", name="lp1")
                    nc.vector.tensor_scalar_add(lp1[:P, :], lg[:P, :], 1.0)
                    fpm = tiny_pool.tile([128, 1], dt, tag="fpm", name="fpm")
                    nc.vector.tensor_tensor(
                        fpm[:P, :], lp1[:P, :], pm[:P, :], op=Alu.mult)
                    nc.vector.tensor_scalar_add(
                        featsT[:P, c * NSPK + n:c * NSPK + n + 1], fpm[:P, :], -1.0)

            # --- phase 2.5: BN1d ---
            f_sum = singles.tile([128, NCH], dt, tag="f_sum", name="f_sum")
            f_sq = singles.tile([128, NCH], dt, tag="f_sq", name="f_sq")
            for c in range(NCH):
                nc.vector.tensor_reduce(
                    f_sum[:, c:c + 1], featsT[:, c * NSPK:(c + 1) * NSPK],
                    axis=Ax.X, op=Alu.add)
                fsq4 = tiny_pool.tile([128, NSPK], dt, tag="fsq4", name="fsq4")
                nc.scalar.activation(
                    fsq4[:], featsT[:, c * NSPK:(c + 1) * NSPK], Act.Square,
                    accum_out=f_sq[:, c:c + 1])
            b1_in = dram.tile([128, 2 * NCH], dt, tag="b1in", name="b1in")
            b1_out = dram.tile([128, 2 * NCH], dt, tag="b1out", name="b1out")
            nc.sync.dma_start(b1_in[:, :NCH], f_sum[:])
            nc.sync.dma_start(b1_in[:, NCH:], f_sq[:])
            nc.gpsimd.collective_compute(
                "AllReduce", mybir.AluOpType.add,
                replica_groups=[list(range(NCORES))],
                ins=[b1_in[:].opt()], outs=[b1_out[:].opt()])
            st1 = singles.tile([128, 2 * NCH], dt)
            nc.sync.dma_start(st1[:], b1_out[:])
            mean1 = tiny_pool.tile([128, NCH], dt, tag="mean1", name="mean1")
            nc.vector.tensor_scalar_mul(mean1[:], st1[:, :NCH], 1.0 / N)
            msq1 = tiny_pool.tile([128, NCH], dt, tag="msq1", name="msq1")
            nc.scalar.activation(msq1[:], mean1[:], Act.Square)
            var1 = tiny_pool.tile([128, NCH], dt, tag="var1", name="var1")
            nc.vector.tensor_scalar_mul(var1[:], st1[:, NCH:], 1.0 / N)
            nc.vector.tensor_tensor(var1[:], var1[:], msq1[:], op=Alu.subtract)
            nc.vector.tensor_scalar_add(var1[:], var1[:], EPS)
            sd1 = tiny_pool.tile([128, NCH], dt, tag="sd1", name="sd1")
            nc.scalar.activation(sd1[:], var1[:], Act.Sqrt)
            rs1 = tiny_pool.tile([128, NCH], dt, tag="rs1", name="rs1")
            nc.vector.reciprocal(rs1[:], sd1[:])
            sb1 = singles.tile([128, NCH], dt, tag="sb1", name="sb1")
            nc.vector.tensor_tensor(sb1[:], rs1[:], bn1g[:], op=Alu.mult)
            tb1 = singles.tile([128, NCH], dt, tag="tb1", name="tb1")
            tm1 = tiny_pool.tile([128, NCH], dt, tag="tm1", name="tm1")
            nc.vector.tensor_tensor(tm1[:], mean1[:], sb1[:], op=Alu.mult)
            nc.vector.tensor_tensor(tb1[:], bn1b[:], tm1[:], op=Alu.subtract)

            # xbnT chunks [128, NSPK] (zero-padded rows already zero via pads)
            xbn = singles.tile([128, NCH * NSPK], dt, tag="xbn", name="xbn")
            nc.vector.memset(xbn[:], 0.0)
            for c, P in enumerate(CHS):
                nc.scalar.activation(
                    xbn[:P, c * NSPK:(c + 1) * NSPK],
                    featsT[:P, c * NSPK:(c + 1) * NSPK], Act.Identity,
                    bias=tb1[:P, c:c + 1], scale=sb1[:P, c:c + 1])

            # --- phase 3: MLP ---
            act = xbn
            bias_sb = []
            for l in range(6):
                bt = singles.tile([128, 8], dt, tag=f"bs{l}", name=f"bs{l}")
                nc.sync.dma_start(bt[:], b_d[l][:])
                bias_sb.append(bt)
            for l in range(6):
                nin_ch = NCH if l == 0 else 8
                wtiles = []
                for jin in range(nin_ch):
                    wt = w_pool.tile([128, HP], dt, tag="wt", name="wt")
                    nc.sync.dma_start(
                        wt[:], w_d[l][128 * jin:128 * (jin + 1), :])
                    wtiles.append(wt)
                out = singles.tile([128, 8 * NSPK], dt, tag=f"h{l}", name=f"h{l}")
                for j in range(8):
                    ps = mlp_ps.tile([128, NSPK], dt, tag="mlpp", name="mlpp")
                    for jin in range(nin_ch):
                        nc.tensor.matmul(
                            ps[:], wtiles[jin][:, 128 * j:128 * (j + 1)],
                            act[:, jin * NSPK:(jin + 1) * NSPK],
                            start=(jin == 0), stop=(jin == nin_ch - 1))
                    nc.scalar.activation(
                        out[:, j * NSPK:(j + 1) * NSPK], ps[:], Act.Relu,
                        bias=bias_sb[l][:, j:j + 1])
                act = out
            # fc7
            w7 = singles.tile([128, 8], dt, tag="w7", name="w7")
            nc.sync.dma_start(
                w7[:], w7_d[:].rearrange("(b a) o -> a (b o)", a=128))
            ps = mlp_ps.tile([128, NSPK], dt, tag="mlpp", name="mlpp")
            for jin in range(8):
                nc.tensor.matmul(
                    ps[:1, :], w7[:, jin:jin + 1],
                    act[:, jin * NSPK:(jin + 1) * NSPK],
                    start=(jin == 0), stop=(jin == 7))
            ysb = singles.tile([128, NSPK], dt, tag="ysb", name="ysb")
            nc.vector.tensor_scalar_add(ysb[:1, :], ps[:1, :], b7_val)
            nc.sync.dma_start(y_out[:, :], ysb[:1, :])

    nc.finalize()
    return nc


_NC_CACHE = {}


def kernel(X1, X2, M1, M2, attn_w,
           bn2d_gamma, bn2d_beta, bn1_gamma, bn1_beta,
           fc1_w, fc1_b, fc2_w, fc2_b, fc3_w, fc3_b, fc4_w, fc4_b,
           fc5_w, fc5_b, fc6_w, fc6_b, fc7_w, fc7_b):
    from concourse.bass_utils import run_bass_kernel_spmd

    fcs = (fc1_w, fc1_b, fc2_w, fc2_b, fc3_w, fc3_b, fc4_w, fc4_b,
           fc5_w, fc5_b, fc6_w, fc6_b, fc7_w, fc7_b)
    (bdz, bds, ident, bn2g, bn2b, bn1g, bn1b,
     wts, w7t, biases, b7v) = _host_prep(
        np.asarray(attn_w, np.float32), np.asarray(bn2d_gamma, np.float32),
        np.asarray(bn2d_beta, np.float32), np.asarray(bn1_gamma, np.float32),
        np.asarray(bn1_beta, np.float32),
        [np.asarray(f, np.float32) for f in fcs])

    key = round(b7v, 10)
    if key not in _NC_CACHE:
        _NC_CACHE[key] = _build_nc(b7v)
    nc = _NC_CACHE[key]

    X1 = np.ascontiguousarray(np.asarray(X1, np.float32).reshape(N, D, V * NF))
    X2 = np.ascontiguousarray(np.asarray(X2, np.float32).reshape(N, D, V * NF))
    M1 = np.ascontiguousarray(np.asarray(M1, np.float32).reshape(N, D, V * NF))
    M2 = np.ascontiguousarray(np.asarray(M2, np.float32).reshape(N, D, V * NF))

    consts = dict(
        bdz=bdz, bds=bds, ident=ident, bn2g=bn2g, bn2b=bn2b,
        bn1g=bn1g, bn1b=bn1b, w7t=w7t,
        **{f"w{l}t": wts[l - 1] for l in range(1, 7)},
        **{f"b{l}": biases[l - 1] for l in range(1, 7)},
    )
    in_maps = []
    for c in range(NCORES):
        sl = slice(NSPK * c, NSPK * (c + 1))
        in_maps.append(dict(
            x1=X1[sl], x2=X2[sl], m1=M1[sl], m2=M2[sl], **consts))

    import os
    trace = bool(int(os.environ.get("KERNEL_TRACE", "0")))
    res = run_bass_kernel_spmd(
        nc, in_maps, core_ids=list(range(NCORES)), trace=trace)
    if res.exec_time_ns is not None:
        print(f"HW exec time: {res.exec_time_ns} ns")
    y = np.concatenate([res.results[c]["y"][0] for c in range(NCORES)])
    return y.astype(np.float32)



# revision 2
# speedup vs baseline: 22437.1999x; 22437.1999x over previous
"""Trainium2 Bass kernel for nn_Deep_Pron (sparse_attention).

Pipeline per core (N-sharded: 4 speakers/core):
  Phase 1: stream X1,X2; per-channel sum/sumsq (BN2d stats) -> AllReduce.
  Phase 1.5: BN2d affine coefs s,t per channel.
  Phase 2: re-stream X + masks; BN-apply (ACT); quadform S via PE
    transpose chunks + blockdiag eigen-matmul + square + blockdiag +/-
    reduce (S lands pair-major [P,100]); softmax; attention output h via
    Pool broadcast-mul + DVE segmented reduce; feats = log||h1-h2||^2.
  Phase 2.5: BN1d stats AllReduce; BN1d apply.
  Phase 3: 7-layer MLP on PE; output y[4] per core.
"""

import numpy as np

N, D, V, NF = 32, 1128, 100, 13
H = 1000
EPS = 1e-5
NCORES = 8
NSPK = N // NCORES  # 4
CHS = [128] * 8 + [104]  # d-chunks per speaker
NCH = len(CHS)
# transpose sub-chunks over the (v,f)=1300 free dim: 11x(9v=117 cols) + 1x(1v=13)
TCH = [(cc * 117, 117, 9) for cc in range(11)] + [(1287, 13, 1)]
CNT2D = float(N * V * NF)  # BN2d count
HP = 1024  # padded H
DP = 1152  # padded D


def _host_prep(attn_w, bn2d_gamma, bn2d_beta, bn1_gamma, bn1_beta, fcs):
    """Build all constant tensors (numpy, f32)."""
    Asym = ((attn_w.T + attn_w) / 2.0).astype(np.float64)
    lam, Q = np.linalg.eigh(Asym)
    B = (Q * np.sqrt(np.abs(lam))[None, :]).astype(np.float32)  # [13,13]
    sign = np.where(lam >= 0, 1.0, -1.0).astype(np.float32)

    bdz = np.zeros((117, 117), np.float32)
    bds = np.zeros((117, 9), np.float32)
    for vp in range(9):
        bdz[13 * vp:13 * vp + 13, 13 * vp:13 * vp + 13] = B
        bds[13 * vp:13 * vp + 13, vp] = sign

    ident = np.eye(128, dtype=np.float32)

    def chunkmajor(vec, pad_val):
        out = np.full((128, NCH), pad_val, np.float32)
        for c, P in enumerate(CHS):
            out[:P, c] = vec[128 * c:128 * c + P]
        return out

    bn2g = chunkmajor(bn2d_gamma, 1.0)
    bn2b = chunkmajor(bn2d_beta, 0.0)
    bn1g = chunkmajor(bn1_gamma, 1.0)
    bn1b = chunkmajor(bn1_beta, 0.0)

    (f1w, f1b, f2w, f2b, f3w, f3b, f4w, f4b, f5w, f5b, f6w, f6b, f7w, f7b) = fcs
    w1t = np.zeros((DP, HP), np.float32)
    w1t[:D, :H] = f1w.T  # [D,H]
    wts = [w1t]
    for w in (f2w, f3w, f4w, f5w, f6w):
        wt = np.zeros((HP, HP), np.float32)
        wt[:H, :H] = w.T
        wts.append(wt)
    w7t = np.zeros((HP, 1), np.float32)
    w7t[:H, 0] = f7w[0]
    biases = []
    for b in (f1b, f2b, f3b, f4b, f5b, f6b):
        bb = np.zeros((128, 8), np.float32)
        for j in range(8):
            seg = b[128 * j:128 * j + 128]
            bb[:len(seg), j] = seg
        biases.append(bb)
    return bdz, bds, ident, bn2g, bn2b, bn1g, bn1b, wts, w7t, biases, float(f7b[0])


def _build_nc(b7_val, level=99):
    import concourse.bass as bass
    import concourse.bacc as bacc
    import concourse.mybir as mybir
    import concourse.tile as tile

    dt = mybir.dt.float32
    Alu = mybir.AluOpType
    Act = mybir.ActivationFunctionType
    Ax = mybir.AxisListType

    nc = bacc.Bacc("TRN2", target_bir_lowering=False, debug=True)

    def din(name, shape):
        return nc.declare_dram_parameter(name, list(shape), dt, isOutput=False)

    x1 = din("x1", (NSPK, D, V * NF))
    x2 = din("x2", (NSPK, D, V * NF))
    m1 = din("m1", (NSPK, D, V * NF))
    m2 = din("m2", (NSPK, D, V * NF))
    bdz_d = din("bdz", (117, 117))
    bds_d = din("bds", (117, 9))
    id_d = din("ident", (128, 128))
    bn2g_d = din("bn2g", (128, NCH))
    bn2b_d = din("bn2b", (128, NCH))
    bn1g_d = din("bn1g", (128, NCH))
    bn1b_d = din("bn1b", (128, NCH))
    w_d = [din(f"w{l}t", (DP if l == 1 else HP, HP)) for l in range(1, 7)]
    w7_d = din("w7t", (HP, 1))
    b_d = [din(f"b{l}", (128, 8)) for l in range(1, 7)]
    y_out = nc.declare_dram_parameter("y", [1, NSPK], dt, isOutput=True)

    xs = (x1, x2)
    ms = (m1, m2)

    with tile.TileContext(nc) as tc:
        with (
            tc.tile_pool(name="singles", bufs=1) as singles,
            tc.tile_pool(name="xin", bufs=3) as xin_pool,
            tc.tile_pool(name="min", bufs=3) as min_pool,
            tc.tile_pool(name="xhat", bufs=2) as xhat_pool,
            tc.tile_pool(name="xt", bufs=4) as xt_pool,
            tc.tile_pool(name="zsq", bufs=4) as zsq_pool,
            tc.tile_pool(name="sm", bufs=4) as sm_pool,
            tc.tile_pool(name="tiny", bufs=8) as tiny_pool,
            tc.tile_pool(name="scratch", bufs=2) as scr_pool,
            tc.tile_pool(name="wpool", bufs=10) as w_pool,
            tc.tile_pool(name="tp_ps", bufs=2, space="PSUM") as tp_ps,
            tc.tile_pool(name="z_ps", bufs=2, space="PSUM") as z_ps,
            tc.tile_pool(name="s_ps", bufs=2, space="PSUM") as s_ps,
            tc.tile_pool(name="mlp_ps", bufs=1, space="PSUM") as mlp_ps,
            tc.tile_pool(name="dram", bufs=1, space="DRAM") as dram,
        ):
            # --- resident constants ---
            ident = singles.tile([128, 128], dt)
            nc.sync.dma_start(ident[:], id_d[:])
            bdz = singles.tile([128, 117], dt)
            nc.sync.dma_start(bdz[:117, :], bdz_d[:])
            bds = singles.tile([128, 9], dt)
            nc.sync.dma_start(bds[:117, :], bds_d[:])
            bn2g = singles.tile([128, NCH], dt)
            nc.sync.dma_start(bn2g[:], bn2g_d[:])
            bn2b = singles.tile([128, NCH], dt)
            nc.sync.dma_start(bn2b[:], bn2b_d[:])
            bn1g = singles.tile([128, NCH], dt)
            nc.sync.dma_start(bn1g[:], bn1g_d[:])
            bn1b = singles.tile([128, NCH], dt)
            nc.sync.dma_start(bn1b[:], bn1b_d[:])

            # --- phase 1: BN2d stats ---
            # acc[xsel]: sum, sumsq tiles [128, NCH]
            acc_sum = [singles.tile([128, NCH], dt, tag=f"acs{i}", name=f"acs{i}") for i in range(2)]
            acc_sq = [singles.tile([128, NCH], dt, tag=f"acq{i}", name=f"acq{i}") for i in range(2)]
            for t in (*acc_sum, *acc_sq):
                nc.vector.memset(t[:], 0.0)

            for n in range(NSPK):
                for c, P in enumerate(CHS):
                    for xi in range(2):
                        xt_ = xin_pool.tile([128, V * NF], dt, tag="p1x", name="p1x")
                        nc.sync.dma_start(
                            xt_[:P, :], xs[xi][n, 128 * c:128 * c + P, :])
                        part = tiny_pool.tile([128, 1], dt, tag="p1part", name="p1part")
                        nc.vector.tensor_reduce(
                            part[:P, :], xt_[:P, :], axis=Ax.X, op=Alu.add)
                        nc.vector.tensor_tensor(
                            acc_sum[xi][:P, c:c + 1], acc_sum[xi][:P, c:c + 1],
                            part[:P, :], op=Alu.add)
                        sq = scr_pool.tile([128, V * NF], dt, tag="p1sq", name="p1sq")
                        sqp = tiny_pool.tile([128, 1], dt, tag="p1sqp", name="p1sqp")
                        nc.scalar.activation(
                            sq[:P, :], xt_[:P, :], Act.Square,
                            accum_out=sqp[:P, :])
                        nc.vector.tensor_tensor(
                            acc_sq[xi][:P, c:c + 1], acc_sq[xi][:P, c:c + 1],
                            sqp[:P, :], op=Alu.add)

            # all-reduce the 4 stat tiles
            st_in = dram.tile([128, 4 * NCH], dt, tag="st_in", name="st_in")
            st_out = dram.tile([128, 4 * NCH], dt, tag="st_out", name="st_out")
            for i in range(2):
                nc.sync.dma_start(st_in[:, NCH * i:NCH * (i + 1)], acc_sum[i][:])
                nc.sync.dma_start(
                    st_in[:, NCH * (2 + i):NCH * (3 + i)], acc_sq[i][:])
            nc.gpsimd.collective_compute(
                "AllReduce", mybir.AluOpType.add,
                replica_groups=[list(range(NCORES))],
                ins=[st_in[:].opt()], outs=[st_out[:].opt()])
            stats = singles.tile([128, 4 * NCH], dt)
            nc.sync.dma_start(stats[:], st_out[:])

            # --- phase 1.5: per-channel affine coefs  s=g*rsqrt(var+eps), t=b-mean*s
            s_co = [singles.tile([128, NCH], dt, tag=f"sco{i}", name=f"sco{i}") for i in range(2)]
            t_co = [singles.tile([128, NCH], dt, tag=f"tco{i}", name=f"tco{i}") for i in range(2)]
            for i in range(2):
                mean = tiny_pool.tile([128, NCH], dt, tag="mean", name="mean")
                nc.vector.tensor_scalar_mul(
                    mean[:], stats[:, NCH * i:NCH * (i + 1)], 1.0 / CNT2D)
                msq = tiny_pool.tile([128, NCH], dt, tag="msq", name="msq")
                nc.scalar.activation(msq[:], mean[:], Act.Square)
                var = tiny_pool.tile([128, NCH], dt, tag="var", name="var")
                nc.vector.tensor_scalar_mul(
                    var[:], stats[:, NCH * (2 + i):NCH * (3 + i)], 1.0 / CNT2D)
                nc.vector.tensor_tensor(var[:], var[:], msq[:], op=Alu.subtract)
                nc.vector.tensor_scalar_add(var[:], var[:], EPS)
                sd = tiny_pool.tile([128, NCH], dt, tag="sd", name="sd")
                nc.scalar.activation(sd[:], var[:], Act.Sqrt)
                rs = tiny_pool.tile([128, NCH], dt, tag="rs", name="rs")
                nc.vector.reciprocal(rs[:], sd[:])
                nc.vector.tensor_tensor(s_co[i][:], rs[:], bn2g[:], op=Alu.mult)
                tm = tiny_pool.tile([128, NCH], dt, tag="tm", name="tm")
                nc.vector.tensor_tensor(tm[:], mean[:], s_co[i][:], op=Alu.mult)
                nc.vector.tensor_tensor(t_co[i][:], bn2b[:], tm[:], op=Alu.subtract)

            # --- phase 2: attention + feats ---
            featsT = singles.tile([128, NCH * NSPK], dt)  # col = c*NSPK+n
            nc.vector.memset(featsT[:], 0.0)

            for n in range(NSPK):
                for c, P in enumerate(CHS):
                    hraw = [None, None]
                    m00 = [None, None]
                    for xi in range(2):
                        xnat = xin_pool.tile([128, V * NF], dt, tag="p2x", name="p2x")
                        nc.sync.dma_start(
                            xnat[:P, :], xs[xi][n, 128 * c:128 * c + P, :])
                        mnat = min_pool.tile([128, V * NF], dt, tag="p2m", name="p2m")
                        nc.sync.dma_start(
                            mnat[:P, :], ms[xi][n, 128 * c:128 * c + P, :])
                        # BN apply
                        xh = xhat_pool.tile([128, V * NF], dt, tag="xh", name="xh")
                        nc.scalar.activation(
                            xh[:P, :], xnat[:P, :], Act.Identity,
                            bias=t_co[xi][:P, c:c + 1], scale=s_co[xi][:P, c:c + 1])
                        # quadform: S pair-major [P, 100]
                        s_psum = s_ps.tile([128, V], dt, tag="spsum", name="spsum")
                        for (off, W, Vc) in TCH:
                            tp = tp_ps.tile([128, 128], dt, tag="tp", name="tp")
                            nc.tensor.transpose(
                                tp[:W, :P], xh[:P, off:off + W], ident[:P, :P])
                            xts = xt_pool.tile([128, 128], dt, tag="xts", name="xts")
                            nc.vector.tensor_copy(xts[:W, :P], tp[:W, :P])
                            zp = z_ps.tile([128, 128], dt, tag="zp", name="zp")
                            nc.tensor.matmul(
                                zp[:W, :P], bdz[:W, :W], xts[:W, :P],
                                start=True, stop=True)
                            zq = zsq_pool.tile([128, 128], dt, tag="zq", name="zq")
                            nc.scalar.activation(zq[:W, :P], zp[:W, :P], Act.Square)
                            vo = off // 13 // 9 * 9
                            nc.tensor.matmul(
                                s_psum[:P, vo:vo + Vc], zq[:W, :P], bds[:W, :Vc],
                                start=True, stop=True)
                        # logits = tanh(S) + 1e5*m0 - 1e5
                        tanh_s = sm_pool.tile([128, V], dt, tag="tanhs", name="tanhs")
                        nc.scalar.activation(
                            tanh_s[:P, :], s_psum[:P, :V], Act.Tanh)
                        mterm = sm_pool.tile([128, V], dt, tag="mterm", name="mterm")
                        m0view = mnat[:P].rearrange("p (v f) -> p v f", f=NF)
                        nc.scalar.activation(
                            mterm[:P, :], m0view[:, :, 0], Act.Copy,
                            scale=1.0e5, bias=-1.0e5)
                        logits = sm_pool.tile([128, V], dt, tag="logits", name="logits")
                        nc.vector.tensor_tensor(
                            logits[:P, :], tanh_s[:P, :], mterm[:P, :], op=Alu.add)
                        # softmax
                        mx = tiny_pool.tile([128, 1], dt, tag="mx", name="mx")
                        nc.vector.tensor_reduce(
                            mx[:P, :], logits[:P, :], axis=Ax.X, op=Alu.max)
                        nmx = tiny_pool.tile([128, 1], dt, tag="nmx", name="nmx")
                        nc.vector.tensor_scalar_mul(nmx[:P, :], mx[:P, :], -1.0)
                        esum = tiny_pool.tile([128, 1], dt, tag="esum", name="esum")
                        ew = sm_pool.tile([128, V], dt, tag="ew", name="ew")
                        nc.scalar.activation(
                            ew[:P, :], logits[:P, :], Act.Exp,
                            bias=nmx[:P, :], accum_out=esum[:P, :])
                        winv = tiny_pool.tile([128, 1], dt, tag="winv", name="winv")
                        nc.vector.reciprocal(winv[:P, :], esum[:P, :])
                        wl3 = sm_pool.tile([128, V], dt, tag="wl3", name="wl3")
                        nc.vector.tensor_scalar_mul(wl3[:P, :], ew[:P, :], winv[:P, :])
                        # h_raw[i] = sum_v W[v] * x[v,i]  (raw x)
                        pall = scr_pool.tile([128, V * NF], dt, tag="pall", name="pall")
                        wb = (wl3[:P, :].rearrange("p (v o) -> p v o", o=1)
                              .broadcast_to((P, V, NF)))
                        xv = xnat[:P].rearrange("p (v f) -> p v f", f=NF)
                        pv = pall[:P].rearrange("p (v f) -> p v f", f=NF)
                        nc.gpsimd.tensor_tensor(pv, xv, wb, op=Alu.mult)
                        hr = tiny_pool.tile([128, NF], dt, tag=f"hr{xi}", name=f"hr{xi}")
                        nc.vector.tensor_reduce(
                            hr[:P, :], pall[:P].rearrange("p (v f) -> p f v", f=NF),
                            axis=Ax.X, op=Alu.add)
                        hraw[xi] = hr
                        mm = tiny_pool.tile([128, 1], dt, tag=f"m00{xi}", name=f"m00{xi}")
                        nc.vector.tensor_copy(mm[:P, :], mnat[:P, 0:1])
                        m00[xi] = mm
                    # feats: g_i = s1*h1_i - s2*h2_i + (t1-t2);  dd = sum g^2
                    g1 = tiny_pool.tile([128, NF], dt, tag="g1", name="g1")
                    nc.vector.tensor_scalar(
                        g1[:P, :], hraw[0][:P, :], s_co[0][:P, c:c + 1],
                        t_co[0][:P, c:c + 1], op0=Alu.mult, op1=Alu.add)
                    g2 = tiny_pool.tile([128, NF], dt, tag="g2", name="g2")
                    nc.vector.tensor_scalar(
                        g2[:P, :], hraw[1][:P, :], s_co[1][:P, c:c + 1],
                        t_co[1][:P, c:c + 1], op0=Alu.mult, op1=Alu.add)
                    gd = tiny_pool.tile([128, NF], dt, tag="gd", name="gd")
                    nc.vector.tensor_tensor(
                        gd[:P, :], g1[:P, :], g2[:P, :], op=Alu.subtract)
                    gsq = tiny_pool.tile([128, NF], dt, tag="gsq", name="gsq")
                    dd = tiny_pool.tile([128, 1], dt, tag="dd", name="dd")
                    nc.scalar.activation(
                        gsq[:P, :], gd[:P, :], Act.Square, accum_out=dd[:P, :])
                    nc.vector.tensor_scalar_add(dd[:P, :], dd[:P, :], EPS)
                    lg = tiny_pool.tile([128, 1], dt, tag="lg", name="lg")
                    nc.scalar.activation(lg[:P, :], dd[:P, :], Act.Ln)
                    pm = tiny_pool.tile([128, 1], dt, tag="pm", name="pm")
                    nc.vector.tensor_tensor(
                        pm[:P, :], m00[0][:P, :], m00[1][:P, :], op=Alu.mult)
                    # feats = (lg+1)*pm - 1
                    lp1 = tiny_pool.tile([128, 1], dt, tag="lp1", name="lp1")
                    nc.vector.tensor_scalar_add(lp1[:P, :], lg[:P, :], 1.0)
                    fpm = tiny_pool.tile([128, 1], dt, tag="fpm", name="fpm")
                    nc.vector.tensor_tensor(
                        fpm[:P, :], lp1[:P, :], pm[:P, :], op=Alu.mult)
                    nc.vector.tensor_scalar_add(
                        featsT[:P, c * NSPK + n:c * NSPK + n + 1], fpm[:P, :], -1.0)

            # --- phase 2.5: BN1d ---
            f_sum = singles.tile([128, NCH], dt, tag="f_sum", name="f_sum")
            f_sq = singles.tile([128, NCH], dt, tag="f_sq", name="f_sq")
            for c in range(NCH):
                nc.vector.tensor_reduce(
                    f_sum[:, c:c + 1], featsT[:, c * NSPK:(c + 1) * NSPK],
                    axis=Ax.X, op=Alu.add)
                fsq4 = tiny_pool.tile([128, NSPK], dt, tag="fsq4", name="fsq4")
                nc.scalar.activation(
                    fsq4[:], featsT[:, c * NSPK:(c + 1) * NSPK], Act.Square,
                    accum_out=f_sq[:, c:c + 1])
            b1_in = dram.tile([128, 2 * NCH], dt, tag="b1in", name="b1in")
            b1_out = dram.tile([128, 2 * NCH], dt, tag="b1out", name="b1out")
            nc.sync.dma_start(b1_in[:, :NCH], f_sum[:])
            nc.sync.dma_start(b1_in[:, NCH:], f_sq[:])
            nc.gpsimd.collective_compute(
                "AllReduce", mybir.AluOpType.add,
                replica_groups=[list(range(NCORES))],
                ins=[b1_in[:].opt()], outs=[b1_out[:].opt()])
            st1 = singles.tile([128, 2 * NCH], dt)
            nc.sync.dma_start(st1[:], b1_out[:])
            mean1 = tiny_pool.tile([128, NCH], dt, tag="mean1", name="mean1")
            nc.vector.tensor_scalar_mul(mean1[:], st1[:, :NCH], 1.0 / N)
            msq1 = tiny_pool.tile([128, NCH], dt, tag="msq1", name="msq1")
            nc.scalar.activation(msq1[:], mean1[:], Act.Square)
            var1 = tiny_pool.tile([128, NCH], dt, tag="var1", name="var1")
            nc.vector.tensor_scalar_mul(var1[:], st1[:, NCH:], 1.0 / N)
            nc.vector.tensor_tensor(var1[:], var1[:], msq1[:], op=Alu.subtract)
            nc.vector.tensor_scalar_add(var1[:], var1[:], EPS)
            sd1 = tiny_pool.tile([128, NCH], dt, tag="sd1", name="sd1")
            nc.scalar.activation(sd1[:], var1[:], Act.Sqrt)
            rs1 = tiny_pool.tile([128, NCH], dt, tag="rs1", name="rs1")
            nc.vector.reciprocal(rs1[:], sd1[:])
            sb1 = singles.tile([128, NCH], dt, tag="sb1", name="sb1")
            nc.vector.tensor_tensor(sb1[:], rs1[:], bn1g[:], op=Alu.mult)
            tb1 = singles.tile([128, NCH], dt, tag="tb1", name="tb1")
            tm1 = tiny_pool.tile([128, NCH], dt, tag="tm1", name="tm1")
            nc.vector.tensor_tensor(tm1[:], mean1[:], sb1[:], op=Alu.mult)
            nc.vector.tensor_tensor(tb1[:], bn1b[:], tm1[:], op=Alu.subtract)

            # xbnT chunks [128, NSPK] (zero-padded rows already zero via pads)
            xbn = singles.tile([128, NCH * NSPK], dt, tag="xbn", name="xbn")
            nc.vector.memset(xbn[:], 0.0)
            for c, P in enumerate(CHS):
                nc.scalar.activation(
                    xbn[:P, c * NSPK:(c + 1) * NSPK],
                    featsT[:P, c * NSPK:(c + 1) * NSPK], Act.Identity,
                    bias=tb1[:P, c:c + 1], scale=sb1[:P, c:c + 1])

            # --- phase 3: MLP ---
            act = xbn
            bias_sb = []
            for l in range(6):
                bt = singles.tile([128, 8], dt, tag=f"bs{l}", name=f"bs{l}")
                nc.sync.dma_start(bt[:], b_d[l][:])
                bias_sb.append(bt)
            for l in range(6):
                nin_ch = NCH if l == 0 else 8
                wtiles = []
                for jin in range(nin_ch):
                    wt = w_pool.tile([128, HP], dt, tag="wt", name="wt")
                    nc.sync.dma_start(
                        wt[:], w_d[l][128 * jin:128 * (jin + 1), :])
                    wtiles.append(wt)
                out = singles.tile([128, 8 * NSPK], dt, tag=f"h{l}", name=f"h{l}")
                for j in range(8):
                    ps = mlp_ps.tile([128, NSPK], dt, tag="mlpp", name="mlpp")
                    for jin in range(nin_ch):
                        nc.tensor.matmul(
                            ps[:], wtiles[jin][:, 128 * j:128 * (j + 1)],
                            act[:, jin * NSPK:(jin + 1) * NSPK],
                            start=(jin == 0), stop=(jin == nin_ch - 1))
                    nc.scalar.activation(
                        out[:, j * NSPK:(j + 1) * NSPK], ps[:], Act.Relu,
                        bias=bias_sb[l][:, j:j + 1])
                act = out
            # fc7
            w7 = singles.tile([128, 8], dt, tag="w7", name="w7")
            nc.sync.dma_start(
                w7[:], w7_d[:].rearrange("(b a) o -> a (b o)", a=128))
            ps = mlp_ps.tile([128, NSPK], dt, tag="mlpp", name="mlpp")
            for jin in range(8):
                nc.tensor.matmul(
                    ps[:1, :], w7[:, jin:jin + 1],
                    act[:, jin * NSPK:(jin + 1) * NSPK],
                    start=(jin == 0), stop=(jin == 7))
            ysb = singles.tile([128, NSPK], dt, tag="ysb", name="ysb")
            nc.vector.tensor_scalar_add(ysb[:1, :], ps[:1, :], b7_val)
            nc.sync.dma_start(y_out[:, :], ysb[:1, :])

    nc.finalize()
    return nc


_NC_CACHE = {}


def kernel(X1, X2, M1, M2, attn_w,
           bn2d_gamma, bn2d_beta, bn1_gamma, bn1_beta,
           fc1_w, fc1_b, fc2_w, fc2_b, fc3_w, fc3_b, fc4_w, fc4_b,
           fc5_w, fc5_b, fc6_w, fc6_b, fc7_w, fc7_b):
    from concourse.bass_utils import run_bass_kernel_spmd

    fcs = (fc1_w, fc1_b, fc2_w, fc2_b, fc3_w, fc3_b, fc4_w, fc4_b,
           fc5_w, fc5_b, fc6_w, fc6_b, fc7_w, fc7_b)
    (bdz, bds, ident, bn2g, bn2b, bn1g, bn1b,
     wts, w7t, biases, b7v) = _host_prep(
        np.asarray(attn_w, np.float32), np.asarray(bn2d_gamma, np.float32),
        np.asarray(bn2d_beta, np.float32), np.asarray(bn1_gamma, np.float32),
        np.asarray(bn1_beta, np.float32),
        [np.asarray(f, np.float32) for f in fcs])

    key = round(b7v, 10)
    if key not in _NC_CACHE:
        _NC_CACHE[key] = _build_nc(b7v)
    nc = _NC_CACHE[key]

    X1 = np.ascontiguousarray(np.asarray(X1, np.float32).reshape(N, D, V * NF))
    X2 = np.ascontiguousarray(np.asarray(X2, np.float32).reshape(N, D, V * NF))
    M1 = np.ascontiguousarray(np.asarray(M1, np.float32).reshape(N, D, V * NF))
    M2 = np.ascontiguousarray(np.asarray(M2, np.float32).reshape(N, D, V * NF))

    consts = dict(
        bdz=bdz, bds=bds, ident=ident, bn2g=bn2g, bn2b=bn2b,
        bn1g=bn1g, bn1b=bn1b, w7t=w7t,
        **{f"w{l}t": wts[l - 1] for l in range(1, 7)},
        **{f"b{l}": biases[l - 1] for l in range(1, 7)},
    )
    in_maps = []
    for c in range(NCORES):
        sl = slice(NSPK * c, NSPK * (c + 1))
        in_maps.append(dict(
            x1=X1[sl], x2=X2[sl], m1=M1[sl], m2=M2[sl], **consts))

    import os
    trace = bool(int(os.environ.get("KERNEL_TRACE", "0")))
    tmpdir = os.environ.get("KERNEL_TMPDIR") or None
    res = run_bass_kernel_spmd(
        nc, in_maps, core_ids=list(range(NCORES)), trace=trace,
        tmpdir=tmpdir)
    if res.exec_time_ns is not None:
        print(f"HW exec time: {res.exec_time_ns} ns")
    y = np.concatenate([res.results[c]["y"][0] for c in range(NCORES)])
    return y.astype(np.float32)



# revision 10
# speedup vs baseline: 35203.8617x; 1.5690x over previous
"""Trainium2 Bass kernel for nn_Deep_Pron (sparse_attention).

Pipeline per core (N-sharded attention, H-sharded MLP):
  Inputs shipped compact: X as float16, masks pre-sliced to M[...,0] float16.
  Phase 1: stream X1,X2 (f16); per-channel sum/sumsq (BN2d stats) -> AllReduce.
  Phase 2: re-stream X + sliced masks; BN-apply; quadform S via PE transpose
    chunks into a wide [117,1536] tile + 3 blockdiag eigen-matmuls + square +
    12 blockdiag signed-reduce matmuls (S pair-major [P,100]); softmax;
    attention output h on BN'd x (Pool broadcast-mul + DVE segmented reduce);
    feats = log||h1-h2||^2.
  Phase 2.5: AllGather feats -> full [32,1128] batch on every core; BN1d local.
  Phase 3: MLP tensor-parallel over H: each core holds a 128-row slice of
    every fc weight, computes its h slice, AllGathers between layers; all
    cores produce the full y[1,32]; host reads core 0.
"""

import os
import numpy as np

N, D, V, NF = 32, 1128, 100, 13
H = 1000
EPS = 1e-5
NCORES = 8
NSPK = N // NCORES  # 4
CHS = [128] * 8 + [104]  # d-chunks per speaker
NCH = len(CHS)
NTCH = 12  # transpose sub-chunks: 11 full (9v x 13f = 117 cols) + 1 tail (1v)
CNT2D = float(N * V * NF)  # BN2d count
HP = 1024  # padded H
DP = 1152  # padded D
F16 = np.float16

import concourse.bass as bass
import concourse.bacc as bacc
import concourse.mybir as mybir
import concourse.tile as tile
from concourse.bass_utils import run_bass_kernel_spmd


def _host_prep(attn_w, bn2d_gamma, bn2d_beta, bn1_gamma, bn1_beta, fcs):
    """Build constant tensors. Weight matrices are per-core H-slices."""
    Asym = ((attn_w.T + attn_w) / 2.0).astype(np.float64)
    lam, Q = np.linalg.eigh(Asym)
    B = (Q * np.sqrt(np.abs(lam))[None, :]).astype(np.float32)  # [13,13]
    sign = np.where(lam >= 0, 1.0, -1.0).astype(np.float32)

    bdz = np.zeros((117, 117), F16)
    bds = np.zeros((117, 9), F16)
    for vp in range(9):
        bdz[13 * vp:13 * vp + 13, 13 * vp:13 * vp + 13] = B.astype(F16)
        bds[13 * vp:13 * vp + 13, vp] = sign.astype(F16)

    ident = np.eye(128, dtype=F16)

    def chunkmajor(vec, pad_val):
        out = np.full((128, NCH), pad_val, np.float32)
        for c, P in enumerate(CHS):
            out[:P, c] = vec[128 * c:128 * c + P]
        return out

    bn2g = chunkmajor(bn2d_gamma, 1.0)
    bn2b = chunkmajor(bn2d_beta, 0.0)
    bn1g = chunkmajor(bn1_gamma, 1.0)
    bn1b = chunkmajor(bn1_beta, 0.0)

    (f1w, f1b, f2w, f2b, f3w, f3b, f4w, f4b, f5w, f5b, f6w, f6b,
     f7w, f7b) = fcs
    # per-core H-slices: core r computes h rows [128r : 128r+128)
    w_slices = []  # w_slices[r] = dict of per-layer lhsT slabs
    b_slices = []
    for r in range(NCORES):
        rs, re = 128 * r, min(128 * (r + 1), H)
        nr = re - rs
        sl = {}
        w1 = np.zeros((DP, 128), np.float32)
        w1[:D, :nr] = f1w[rs:re, :].T
        sl["w1s"] = w1
        for l, w in zip(range(2, 7), (f2w, f3w, f4w, f5w, f6w)):
            wt = np.zeros((HP, 128), np.float32)
            wt[:H, :nr] = w[rs:re, :].T
            sl[f"w{l}s"] = wt
        w_slices.append(sl)
        bb = {}
        for l, b in zip(range(1, 7), (f1b, f2b, f3b, f4b, f5b, f6b)):
            bcol = np.zeros((128, 1), np.float32)
            bcol[:nr, 0] = b[rs:re]
            bb[f"b{l}s"] = bcol
        b_slices.append(bb)
    w7t = np.zeros((HP, 1), np.float32)
    w7t[:H, 0] = f7w[0]
    b7 = np.full((1, 1), float(f7b[0]), np.float32)
    return bdz, bds, ident, bn2g, bn2b, bn1g, bn1b, w_slices, b_slices, w7t, b7


def _build_nc():
    dt = mybir.dt.float32
    f16 = mybir.dt.float16
    Alu = mybir.AluOpType
    Act = mybir.ActivationFunctionType
    Ax = mybir.AxisListType

    nc = bacc.Bacc("TRN2", target_bir_lowering=False, debug=True)

    def din(name, shape, ty=dt):
        return nc.declare_dram_parameter(name, list(shape), ty, isOutput=False)

    x1 = din("x1", (NSPK, D, V * NF), f16)
    x2 = din("x2", (NSPK, D, V * NF), f16)
    m1 = din("m1", (NSPK, D, V), f16)
    m2 = din("m2", (NSPK, D, V), f16)
    bdz_d = din("bdz", (117, 117), f16)
    bds_d = din("bds", (117, 9), f16)
    id_d = din("ident", (128, 128), f16)
    bn2g_d = din("bn2g", (128, NCH))
    bn2b_d = din("bn2b", (128, NCH))
    bn1g_d = din("bn1g", (128, NCH))
    bn1b_d = din("bn1b", (128, NCH))
    w_d = {1: din("w1s", (DP, 128))}
    for l in range(2, 7):
        w_d[l] = din(f"w{l}s", (HP, 128))
    w7_d = din("w7t", (HP, 1))
    b_d = {l: din(f"b{l}s", (128, 1)) for l in range(1, 7)}
    b7_d = din("b7", (1, 1))
    y_out = nc.declare_dram_parameter("y", [1, N], dt, isOutput=True)

    xs = (x1, x2)
    ms = (m1, m2)
    grp = [list(range(NCORES))]

    with tile.TileContext(nc) as tc:
        with (
            tc.tile_pool(name="singles", bufs=1) as singles,
            tc.tile_pool(name="xin", bufs=3) as xin_pool,
            tc.tile_pool(name="min", bufs=3) as min_pool,
            tc.tile_pool(name="xhat", bufs=2) as xhat_pool,
            tc.tile_pool(name="xt", bufs=2) as xt_pool,
            tc.tile_pool(name="zsq", bufs=2) as zsq_pool,
            tc.tile_pool(name="sm", bufs=4) as sm_pool,
            tc.tile_pool(name="tiny", bufs=8) as tiny_pool,
            tc.tile_pool(name="scratch", bufs=2) as scr_pool,
            tc.tile_pool(name="wpool", bufs=10) as w_pool,
            tc.tile_pool(name="tp_ps", bufs=2, space="PSUM") as tp_ps,
            tc.tile_pool(name="z_ps", bufs=2, space="PSUM") as z_ps,
            tc.tile_pool(name="s_ps", bufs=2, space="PSUM") as s_ps,
            tc.tile_pool(name="mlp_ps", bufs=2, space="PSUM") as mlp_ps,
            tc.tile_pool(name="dram", bufs=1, space="DRAM") as dram,
        ):
            # --- resident constants ---
            ident = singles.tile([128, 128], f16)
            nc.sync.dma_start(ident[:], id_d[:])
            bdz = singles.tile([128, 117], f16)
            nc.sync.dma_start(bdz[:117, :], bdz_d[:])
            bds = singles.tile([128, 9], f16)
            nc.sync.dma_start(bds[:117, :], bds_d[:])
            bn2g = singles.tile([128, NCH], dt)
            nc.sync.dma_start(bn2g[:], bn2g_d[:])
            bn2b = singles.tile([128, NCH], dt)
            nc.sync.dma_start(bn2b[:], bn2b_d[:])
            bn1g = singles.tile([128, NCH], dt)
            nc.sync.dma_start(bn1g[:], bn1g_d[:])
            bn1b = singles.tile([128, NCH], dt)
            nc.sync.dma_start(bn1b[:], bn1b_d[:])

            # --- phase 1: BN2d stats ---
            acc_sum = [singles.tile([128, NCH], dt, tag=f"acs{i}", name=f"acs{i}") for i in range(2)]
            acc_sq = [singles.tile([128, NCH], dt, tag=f"acq{i}", name=f"acq{i}") for i in range(2)]
            for t in (*acc_sum, *acc_sq):
                nc.vector.memset(t[:], 0.0)

            for n in range(NSPK):
                for c, P in enumerate(CHS):
                    for xi in range(2):
                        xt_ = xin_pool.tile([128, V * NF], f16, tag="p1x", name="p1x")
                        nc.sync.dma_start(
                            xt_[:P, :], xs[xi][n, 128 * c:128 * c + P, :])
                        part = tiny_pool.tile([128, 1], dt, tag="p1part", name="p1part")
                        nc.vector.tensor_reduce(
                            part[:P, :], xt_[:P, :], axis=Ax.X, op=Alu.add)
                        nc.vector.tensor_tensor(
                            acc_sum[xi][:P, c:c + 1], acc_sum[xi][:P, c:c + 1],
                            part[:P, :], op=Alu.add)
                        sq = scr_pool.tile([128, V * NF], f16, tag="p1sq", name="p1sq")
                        sqp = tiny_pool.tile([128, 1], dt, tag="p1sqp", name="p1sqp")
                        nc.scalar.activation(
                            sq[:P, :], xt_[:P, :], Act.Square,
                            accum_out=sqp[:P, :])
                        nc.vector.tensor_tensor(
                            acc_sq[xi][:P, c:c + 1], acc_sq[xi][:P, c:c + 1],
                            sqp[:P, :], op=Alu.add)

            # all-reduce the 4 stat tiles
            st_in = dram.tile([128, 4 * NCH], dt, tag="st_in", name="st_in")
            st_out = dram.tile([128, 4 * NCH], dt, tag="st_out", name="st_out")
            for i in range(2):
                nc.sync.dma_start(st_in[:, NCH * i:NCH * (i + 1)], acc_sum[i][:])
                nc.sync.dma_start(
                    st_in[:, NCH * (2 + i):NCH * (3 + i)], acc_sq[i][:])
            nc.gpsimd.collective_compute(
                "AllReduce", mybir.AluOpType.add,
                replica_groups=grp,
                ins=[st_in[:].opt()], outs=[st_out[:].opt()])
            stats = singles.tile([128, 4 * NCH], dt)
            nc.sync.dma_start(stats[:], st_out[:])

            # --- phase 1.5: per-channel affine coefs s=g*rsqrt(var+eps), t=b-mean*s
            s_co = [singles.tile([128, NCH], dt, tag=f"sco{i}", name=f"sco{i}") for i in range(2)]
            t_co = [singles.tile([128, NCH], dt, tag=f"tco{i}", name=f"tco{i}") for i in range(2)]
            for i in range(2):
                mean = tiny_pool.tile([128, NCH], dt, tag="mean", name="mean")
                nc.vector.tensor_scalar_mul(
                    mean[:], stats[:, NCH * i:NCH * (i + 1)], 1.0 / CNT2D)
                msq = tiny_pool.tile([128, NCH], dt, tag="msq", name="msq")
                nc.scalar.activation(msq[:], mean[:], Act.Square)
                var = tiny_pool.tile([128, NCH], dt, tag="var", name="var")
                nc.vector.tensor_scalar_mul(
                    var[:], stats[:, NCH * (2 + i):NCH * (3 + i)], 1.0 / CNT2D)
                nc.vector.tensor_tensor(var[:], var[:], msq[:], op=Alu.subtract)
                nc.vector.tensor_scalar_add(var[:], var[:], EPS)
                sd = tiny_pool.tile([128, NCH], dt, tag="sd", name="sd")
                nc.scalar.activation(sd[:], var[:], Act.Sqrt)
                rs = tiny_pool.tile([128, NCH], dt, tag="rs", name="rs")
                nc.vector.reciprocal(rs[:], sd[:])
                nc.vector.tensor_tensor(s_co[i][:], rs[:], bn2g[:], op=Alu.mult)
                tm = tiny_pool.tile([128, NCH], dt, tag="tm", name="tm")
                nc.vector.tensor_tensor(tm[:], mean[:], s_co[i][:], op=Alu.mult)
                nc.vector.tensor_tensor(t_co[i][:], bn2b[:], tm[:], op=Alu.subtract)

            # --- phase 2: attention + feats ---
            featsT = singles.tile([128, NCH * NSPK], dt)  # col = c*NSPK+n
            nc.vector.memset(featsT[:], 0.0)

            for n in range(NSPK):
                for c, P in enumerate(CHS):
                    hhat = [None, None]
                    m00 = [None, None]
                    for xi in range(2):
                        xnat = xin_pool.tile([128, V * NF], f16, tag="p2x", name="p2x")
                        nc.sync.dma_start(
                            xnat[:P, :], xs[xi][n, 128 * c:128 * c + P, :])
                        mnat = min_pool.tile([128, V], f16, tag="p2m", name="p2m")
                        nc.sync.dma_start(
                            mnat[:P, :], ms[xi][n, 128 * c:128 * c + P, :])
                        # BN apply -> xh (f16)
                        xh = xhat_pool.tile([128, V * NF], f16, tag="xh", name="xh")
                        nc.scalar.activation(
                            xh[:P, :], xnat[:P, :], Act.Identity,
                            bias=t_co[xi][:P, c:c + 1], scale=s_co[xi][:P, c:c + 1])
                        # transpose chunks into wide tile: col block cc = d,
                        # partition = (vg, f) for v = 9*cc + vg
                        xts = xt_pool.tile([128, NTCH * 128], f16, tag="xts", name="xts")
                        # tail chunk covers only v=99: zero rows 13:117 there
                        nc.vector.memset(xts[:117, 11 * 128:], 0.0)
                        for cc in range(NTCH):
                            off = cc * 117
                            W = 117 if cc < 11 else 13
                            tp = tp_ps.tile([128, 128], f16, tag="tp", name="tp")
                            nc.tensor.transpose(
                                tp[:W, :P], xh[:P, off:off + W], ident[:P, :P])
                            nc.vector.tensor_copy(
                                xts[:W, cc * 128:cc * 128 + P], tp[:W, :P])
                        # z = blockdiag(B)^T-ish eigen transform, 3 wide matmuls
                        zq = zsq_pool.tile([128, NTCH * 128], f16, tag="zq", name="zq")
                        for k in range(3):
                            co = k * 512
                            zp = z_ps.tile([128, 512], dt, tag="zp", name="zp")
                            nc.tensor.matmul(
                                zp[:117, :], bdz[:117, :117], xts[:117, co:co + 512],
                                start=True, stop=True)
                            nc.scalar.activation(
                                zq[:117, co:co + 512], zp[:117, :], Act.Square)
                        # signed reduce over f: S pair-major [P, 100]
                        s_psum = s_ps.tile([128, 100], dt, tag="spsum", name="spsum")
                        for cc in range(NTCH):
                            W = 117 if cc < 11 else 13
                            Vc = 9 if cc < 11 else 1
                            nc.tensor.matmul(
                                s_psum[:P, 9 * cc:9 * cc + Vc],
                                zq[:W, cc * 128:cc * 128 + P], bds[:W, :Vc],
                                start=True, stop=True)
                        # logits = tanh(S) + 1e5*m0 - 1e5
                        tanh_s = sm_pool.tile([128, V], dt, tag="tanhs", name="tanhs")
                        nc.scalar.activation(
                            tanh_s[:P, :], s_psum[:P, :V], Act.Tanh)
                        mterm = sm_pool.tile([128, V], dt, tag="mterm", name="mterm")
                        nc.scalar.activation(
                            mterm[:P, :], mnat[:P, :], Act.Copy,
                            scale=1.0e5, bias=-1.0e5)
                        logits = sm_pool.tile([128, V], dt, tag="logits", name="logits")
                        nc.vector.tensor_tensor(
                            logits[:P, :], tanh_s[:P, :], mterm[:P, :], op=Alu.add)
                        # softmax
                        mx = tiny_pool.tile([128, 1], dt, tag="mx", name="mx")
                        nc.vector.tensor_reduce(
                            mx[:P, :], logits[:P, :], axis=Ax.X, op=Alu.max)
                        nmx = tiny_pool.tile([128, 1], dt, tag="nmx", name="nmx")
                        nc.vector.tensor_scalar_mul(nmx[:P, :], mx[:P, :], -1.0)
                        esum = tiny_pool.tile([128, 1], dt, tag="esum", name="esum")
                        ew = sm_pool.tile([128, V], dt, tag="ew", name="ew")
                        nc.scalar.activation(
                            ew[:P, :], logits[:P, :], Act.Exp,
                            bias=nmx[:P, :], accum_out=esum[:P, :])
                        winv = tiny_pool.tile([128, 1], dt, tag="winv", name="winv")
                        nc.vector.reciprocal(winv[:P, :], esum[:P, :])
                        wl3 = sm_pool.tile([128, V], dt, tag="wl3", name="wl3")
                        nc.vector.tensor_scalar_mul(wl3[:P, :], ew[:P, :], winv[:P, :])
                        # h_hat[i] = sum_v W[v] * xh[v,i]  (BN'd x: matches ref)
                        pall = scr_pool.tile([128, V * NF], dt, tag="pall", name="pall")
                        wb = (wl3[:P, :].rearrange("p (v o) -> p v o", o=1)
                              .broadcast_to((P, V, NF)))
                        xv = xh[:P].rearrange("p (v f) -> p v f", f=NF)
                        pv = pall[:P].rearrange("p (v f) -> p v f", f=NF)
                        nc.gpsimd.tensor_tensor(pv, xv, wb, op=Alu.mult)
                        hh = tiny_pool.tile([128, NF], dt, tag=f"hh{xi}", name=f"hh{xi}")
                        nc.vector.tensor_reduce(
                            hh[:P, :], pall[:P].rearrange("p (v f) -> p f v", f=NF),
                            axis=Ax.X, op=Alu.add)
                        hhat[xi] = hh
                        mm = tiny_pool.tile([128, 1], dt, tag=f"m00{xi}", name=f"m00{xi}")
                        nc.vector.tensor_copy(mm[:P, :], mnat[:P, 0:1])
                        m00[xi] = mm
                    # feats: dd = sum (h1hat - h2hat)^2
                    gd = tiny_pool.tile([128, NF], dt, tag="gd", name="gd")
                    nc.vector.tensor_tensor(
                        gd[:P, :], hhat[0][:P, :], hhat[1][:P, :], op=Alu.subtract)
                    gsq = tiny_pool.tile([128, NF], dt, tag="gsq", name="gsq")
                    dd = tiny_pool.tile([128, 1], dt, tag="dd", name="dd")
                    nc.scalar.activation(
                        gsq[:P, :], gd[:P, :], Act.Square, accum_out=dd[:P, :])
                    nc.vector.tensor_scalar_add(dd[:P, :], dd[:P, :], EPS)
                    lg = tiny_pool.tile([128, 1], dt, tag="lg", name="lg")
                    nc.scalar.activation(lg[:P, :], dd[:P, :], Act.Ln)
                    pm = tiny_pool.tile([128, 1], dt, tag="pm", name="pm")
                    nc.vector.tensor_tensor(
                        pm[:P, :], m00[0][:P, :], m00[1][:P, :], op=Alu.mult)
                    # feats = (lg+1)*pm - 1
                    lp1 = tiny_pool.tile([128, 1], dt, tag="lp1", name="lp1")
                    nc.vector.tensor_scalar_add(lp1[:P, :], lg[:P, :], 1.0)
                    fpm = tiny_pool.tile([128, 1], dt, tag="fpm", name="fpm")
                    nc.vector.tensor_tensor(
                        fpm[:P, :], lp1[:P, :], pm[:P, :], op=Alu.mult)
                    nc.vector.tensor_scalar_add(
                        featsT[:P, c * NSPK + n:c * NSPK + n + 1], fpm[:P, :], -1.0)

            # --- phase 2.5: AllGather feats -> full batch; BN1d locally ---
            fg_in = dram.tile([128, NCH * NSPK], dt, tag="fg_in", name="fg_in")
            fg_out = dram.tile([NCORES, 128 * NCH * NSPK], dt, tag="fg_out", name="fg_out")
            nc.sync.dma_start(fg_in[:], featsT[:])
            nc.gpsimd.collective_compute(
                "AllGather", mybir.AluOpType.bypass,
                replica_groups=grp,
                ins=[fg_in[:].opt()], outs=[fg_out[:].opt()])
            # gathered layout: [k, p, (c, n)] -> sbuf [p, (k, c, n)]
            gatf = singles.tile([128, NCORES * NCH * NSPK], dt)
            nc.sync.dma_start(
                gatf[:].rearrange("p (k q) -> p k q", k=NCORES),
                fg_out[:].rearrange("k (p q) -> p k q", p=128))
            # compact to c-major speaker layout xg[p, (c, k, n)]; sp = 4k+n
            xg = singles.tile([128, NCH * N], dt, tag="xg", name="xg")
            gv = gatf[:].rearrange("p (k c n) -> p c k n", c=NCH, n=NSPK)
            for c in range(NCH):
                nc.vector.tensor_copy(
                    xg[:, c * N:(c + 1) * N].rearrange(
                        "p (k n) -> p k n", k=NCORES),
                    gv[:, c])
            # BN1d stats over the full batch (local now)
            f_sum = singles.tile([128, NCH], dt, tag="f_sum", name="f_sum")
            f_sq = singles.tile([128, NCH], dt, tag="f_sq", name="f_sq")
            for c in range(NCH):
                nc.vector.tensor_reduce(
                    f_sum[:, c:c + 1], xg[:, c * N:(c + 1) * N],
                    axis=Ax.X, op=Alu.add)
                fsqs = tiny_pool.tile([128, N], dt, tag="fsqs", name="fsqs")
                nc.scalar.activation(
                    fsqs[:], xg[:, c * N:(c + 1) * N], Act.Square,
                    accum_out=f_sq[:, c:c + 1])
            mean1 = tiny_pool.tile([128, NCH], dt, tag="mean1", name="mean1")
            nc.vector.tensor_scalar_mul(mean1[:], f_sum[:], 1.0 / N)
            msq1 = tiny_pool.tile([128, NCH], dt, tag="msq1", name="msq1")
            nc.scalar.activation(msq1[:], mean1[:], Act.Square)
            var1 = tiny_pool.tile([128, NCH], dt, tag="var1", name="var1")
            nc.vector.tensor_scalar_mul(var1[:], f_sq[:], 1.0 / N)
            nc.vector.tensor_tensor(var1[:], var1[:], msq1[:], op=Alu.subtract)
            nc.vector.tensor_scalar_add(var1[:], var1[:], EPS)
            sd1 = tiny_pool.tile([128, NCH], dt, tag="sd1", name="sd1")
            nc.scalar.activation(sd1[:], var1[:], Act.Sqrt)
            rs1 = tiny_pool.tile([128, NCH], dt, tag="rs1", name="rs1")
            nc.vector.reciprocal(rs1[:], sd1[:])
            sb1 = singles.tile([128, NCH], dt, tag="sb1", name="sb1")
            nc.vector.tensor_tensor(sb1[:], rs1[:], bn1g[:], op=Alu.mult)
            tb1 = singles.tile([128, NCH], dt, tag="tb1", name="tb1")
            tm1 = tiny_pool.tile([128, NCH], dt, tag="tm1", name="tm1")
            nc.vector.tensor_tensor(tm1[:], mean1[:], sb1[:], op=Alu.mult)
            nc.vector.tensor_tensor(tb1[:], bn1b[:], tm1[:], op=Alu.subtract)

            # xbn [p, (c, sp)] zero-padded rows
            xbn = singles.tile([128, NCH * N], dt, tag="xbn", name="xbn")
            nc.vector.memset(xbn[:], 0.0)
            for c, P in enumerate(CHS):
                nc.scalar.activation(
                    xbn[:P, c * N:(c + 1) * N],
                    xg[:P, c * N:(c + 1) * N], Act.Identity,
                    bias=tb1[:P, c:c + 1], scale=sb1[:P, c:c + 1])

            # --- phase 3: MLP, H-sharded; AllGather h between layers ---
            bias_sb = {}
            for l in range(1, 7):
                bt = singles.tile([128, 1], dt, tag=f"bs{l}", name=f"bs{l}")
                nc.sync.dma_start(bt[:], b_d[l][:])
                bias_sb[l] = bt
            b7sb = singles.tile([128, 1], dt, tag="b7sb", name="b7sb")
            nc.sync.dma_start(b7sb[:1, :], b7_d[:])

            act_full = xbn  # [128, nin_ch * N]
            for l in range(1, 7):
                nin_ch = NCH if l == 1 else NCORES
                wtiles = []
                for g in range(nin_ch):
                    wt = w_pool.tile([128, 128], dt, tag="wt", name="wt")
                    nc.sync.dma_start(
                        wt[:], w_d[l][128 * g:128 * (g + 1), :])
                    wtiles.append(wt)
                ps = mlp_ps.tile([128, 32], dt, tag="mlpp", name="mlpp")
                for g in range(nin_ch):
                    nc.tensor.matmul(
                        ps[:, :N], wtiles[g][:],
                        act_full[:, g * N:(g + 1) * N],
                        start=(g == 0), stop=(g == nin_ch - 1))
                hloc = singles.tile([128, N], dt, tag=f"h{l}", name=f"h{l}")
                nc.scalar.activation(
                    hloc[:, :], ps[:, :N], Act.Relu, bias=bias_sb[l][:, 0:1])
                # AllGather h slices -> full activation on every core
                hg_in = dram.tile([128, N], dt, tag=f"hgi{l}", name=f"hgi{l}")
                hg_out = dram.tile([NCORES, 128 * N], dt, tag=f"hgo{l}", name=f"hgo{l}")
                nc.sync.dma_start(hg_in[:], hloc[:])
                nc.gpsimd.collective_compute(
                    "AllGather", mybir.AluOpType.bypass,
                    replica_groups=grp,
                    ins=[hg_in[:].opt()], outs=[hg_out[:].opt()])
                gath = singles.tile([128, NCORES * N], dt, tag=f"gh{l}", name=f"gh{l}")
                nc.sync.dma_start(
                    gath[:].rearrange("p (k q) -> p k q", k=NCORES),
                    hg_out[:].rearrange("k (p q) -> p k q", p=128))
                act_full = gath
            # fc7: every core computes full y[1, 32]
            w7 = singles.tile([128, 8], dt, tag="w7", name="w7")
            nc.sync.dma_start(
                w7[:], w7_d[:].rearrange("(b a) o -> a (b o)", a=128))
            ps7 = mlp_ps.tile([128, 32], dt, tag="mlpp", name="ps7")
            for g in range(NCORES):
                nc.tensor.matmul(
                    ps7[:1, :N], w7[:, g:g + 1],
                    act_full[:, g * N:(g + 1) * N],
                    start=(g == 0), stop=(g == NCORES - 1))
            ysb = singles.tile([128, N], dt, tag="ysb", name="ysb")
            nc.scalar.activation(
                ysb[:1, :], ps7[:1, :N], Act.Identity, bias=b7sb[:1, 0:1])
            nc.sync.dma_start(y_out[:, :], ysb[:1, :])

    nc.finalize()
    return nc


_NC = _build_nc()


def kernel(X1, X2, M1, M2, attn_w,
           bn2d_gamma, bn2d_beta, bn1_gamma, bn1_beta,
           fc1_w, fc1_b, fc2_w, fc2_b, fc3_w, fc3_b, fc4_w, fc4_b,
           fc5_w, fc5_b, fc6_w, fc6_b, fc7_w, fc7_b):
    fcs = [np.asarray(f, np.float32) for f in (
        fc1_w, fc1_b, fc2_w, fc2_b, fc3_w, fc3_b, fc4_w, fc4_b,
        fc5_w, fc5_b, fc6_w, fc6_b, fc7_w, fc7_b)]
    (bdz, bds, ident, bn2g, bn2b, bn1g, bn1b,
     w_slices, b_slices, w7t, b7) = _host_prep(
        np.asarray(attn_w, np.float32), np.asarray(bn2d_gamma, np.float32),
        np.asarray(bn2d_beta, np.float32), np.asarray(bn1_gamma, np.float32),
        np.asarray(bn1_beta, np.float32), fcs)

    X1 = np.asarray(X1).reshape(N, D, V * NF).astype(F16)
    X2 = np.asarray(X2).reshape(N, D, V * NF).astype(F16)
    M1 = np.asarray(M1).reshape(N, D, V, NF)[:, :, :, 0].astype(F16)
    M2 = np.asarray(M2).reshape(N, D, V, NF)[:, :, :, 0].astype(F16)

    consts = dict(
        bdz=bdz, bds=bds, ident=ident, bn2g=bn2g, bn2b=bn2b,
        bn1g=bn1g, bn1b=bn1b, w7t=w7t, b7=b7,
    )
    in_maps = []
    for c in range(NCORES):
        sl = slice(NSPK * c, NSPK * (c + 1))
        in_maps.append(dict(
            x1=X1[sl], x2=X2[sl], m1=M1[sl], m2=M2[sl],
            **w_slices[c], **b_slices[c], **consts))

    trace = bool(int(os.environ.get("KERNEL_TRACE", "0")))
    tmpdir = os.environ.get("KERNEL_TMPDIR") or None
    res = run_bass_kernel_spmd(
        _NC, in_maps, core_ids=list(range(NCORES)), trace=trace,
        tmpdir=tmpdir)
    if res.exec_time_ns is not None:
        print(f"HW exec time: {res.exec_time_ns} ns")
    y = res.results[0]["y"][0]
    return y.astype(np.float32)
